# revision 12
# baseline (speedup 1.0000x reference)
"""Trainium2 Bass kernel for nn_PyrDown (masked 5x5 Gaussian blur + antialiased
2x bilinear downsample), data-parallel over batch across 8 NeuronCores.

Math per image (reflect-pad 2):
  num = conv2d_5x5(x*m), den = conv2d_5x5(m)  (unnormalized integer binomial
  weights; den weights additionally scaled by 64 to fold the pooling /8/8)
  blur = num * (1/den64)
  out  = antialiased 2x downsample (jax.image.resize antialias=True):
         separable [1,3,3,1]/8 stride 2 with edge renormalization (the /8 is
         folded into den64, edge /7-renorm applied on host as x8/7)

Device mapping per H-slab (M blur rows = P pooled rows * 2 + halo, K = M+4
input rows, even blur rows in low partitions / odd in high -- the DMA loads
rows even/odd-interleaved and the banded weight matrices absorb the
permutation):
  - 4 fp16 DMA loads (x evens/odds, m evens/odds)
  - DVE: t = x*m (fp16)
  - PE: direct 2D conv: per 512-col half, 5 accumulating fp16 matmuls each for
    num and den (H-taps in the banded stationary weights, W-taps via shifted
    rhs slices)
  - DVE: rden = reciprocal_approx_fast(den) (fp32), blur = num*rden (fp16)
  - GPS: W-pool pair sums s1, s2 (stride-2 fp16); DVE: wp4 = 3*s1 + s2
  - PE: H-pool matmul php = Pmat^T @ wp4 ([1,3,3,1] banded, stride-2)
  - ACT: fp16 evac of php; DMA out
"""
import sys
import numpy as np

_TRN_REPO = "/opt/trn_rl_repo"
if _TRN_REPO not in sys.path:
    sys.path.insert(0, _TRN_REPO)

# Problem shape (hardcoded per spec)
B, C, H, W = 16, 3, 1024, 1024
N_CORES = 8

KH = np.array([1.0, 4.0, 6.0, 4.0, 1.0], np.float32)


# ---------------------------------------------------------------- slab plan
class Slab:
    pass


def slab_plan(H):
    """Split the image into H-slabs. Each slab computes P pooled rows, which
    need M blur rows (M even, includes +-1 halo), which need K = M+4 padded
    rows. Partition layout of blur rows: even rows first, then odd rows."""
    HO = H // 2
    slabs = []
    o0 = 0
    while o0 < HO:
        P = min(61, HO - o0)
        b_lo = max(0, 2 * o0 - 1)
        b_hi = min(H - 1, 2 * (o0 + P - 1) + 2)
        if (b_hi - b_lo + 1) % 2:
            if b_lo > 0:
                b_lo -= 1
            else:
                b_hi += 1
                assert b_hi <= H - 1
        M = b_hi - b_lo + 1
        K = M + 4
        assert K <= 128 and b_lo + K <= H + 4
        evens = [b for b in range(b_lo, b_lo + M) if b % 2 == 0]
        odds = [b for b in range(b_lo, b_lo + M) if b % 2 == 1]
        out_rows = evens + odds  # partition order of blur rows
        pos = {r: i for i, r in enumerate(out_rows)}

        s = Slab()
        s.o0, s.P, s.b_lo, s.M, s.K = o0, P, b_lo, M, K
        s.r0 = b_lo  # first padded row to load
        s.out_rows = out_rows
        # input padded rows, partition order (even/odd offset interleave)
        s.in_rows = [s.r0 + 2 * p for p in range(K // 2)] + [
            s.r0 + 2 * p + 1 for p in range(K // 2)
        ]
        # H-pool matrix [M, P]: pooled row o0+q <- blur rows 2o-1,2o,2o+1,2o+2
        # with weights 1,3,3,1; out-of-range taps dropped (host renormalizes
        # the image's first/last pooled row by 8/7).
        Pm = np.zeros((M, P), np.float32)
        for q in range(P):
            o = o0 + q
            for (r, w) in [(2 * o - 1, 1.0), (2 * o, 3.0),
                           (2 * o + 1, 3.0), (2 * o + 2, 1.0)]:
                if 0 <= r <= H - 1:
                    Pm[pos[r], q] = w
        s.Pmat = Pm
        slabs.append(s)
        o0 += P
    return slabs


def build_weight_arrays(slabs):
    """Per distinct slab geometry: conv weights [K, 10*M] fp16 (5 W-taps for
    num, then 5 scaled by 64 for den) and the H-pool matrix [M, P] fp16."""
    arrays = {}
    for s in slabs:
        key = (s.K, s.M, s.o0 == 0, tuple(r - s.r0 for r in s.out_rows))
        s.wkey = key
        if key in arrays:
            continue
        Wt = np.zeros((s.K, 10 * s.M), np.float32)
        for p in range(s.K):
            ir = s.in_rows[p]
            for m in range(s.M):
                d = ir - s.out_rows[m]
                if 0 <= d <= 4:
                    for tap in range(5):
                        Wt[p, tap * s.M + m] = KH[tap] * KH[d]
                        Wt[p, (5 + tap) * s.M + m] = 64.0 * KH[tap] * KH[d]
        arrays[key] = (Wt.astype(np.float16), s.Pmat.astype(np.float16))
    return arrays


# ---------------------------------------------------------------- numpy model
def numpy_model(x, m, H, W, fp16=True):
    """Model of exactly what the device computes (optionally with fp16
    rounding of the conv inputs), plus the host edge fixup."""
    slabs = slab_plan(H)
    WO = W // 2
    xp = np.pad(x, 2, mode="reflect")
    mp = np.pad(m, 2, mode="reflect")
    if fp16:
        xp = xp.astype(np.float16).astype(np.float32)
        mp = mp.astype(np.float16).astype(np.float32)
    tp = (xp * mp)
    if fp16:
        tp = tp.astype(np.float16).astype(np.float32)
    out = np.zeros((H // 2, WO), np.float32)
    for s in slabs:
        t_sb = tp[s.in_rows, :]
        m_sb = mp[s.in_rows, :]
        Wt = np.zeros((s.K, 10 * s.M), np.float32)
        for p in range(s.K):
            ir = s.in_rows[p]
            for mm in range(s.M):
                d = ir - s.out_rows[mm]
                if 0 <= d <= 4:
                    for tap in range(5):
                        Wt[p, tap * s.M + mm] = KH[tap] * KH[d]
                        Wt[p, (5 + tap) * s.M + mm] = 64.0 * KH[tap] * KH[d]
        pnum = np.zeros((s.M, W), np.float32)
        pden = np.zeros((s.M, W), np.float32)
        for tap in range(5):
            pnum += Wt[:, tap * s.M:(tap + 1) * s.M].T @ t_sb[:, tap:tap + W]
            pden += Wt[:, (5 + tap) * s.M:(6 + tap) * s.M].T @ m_sb[:, tap:tap + W]
        rden = 1.0 / pden
        blur = pnum * rden
        if fp16:
            blur = blur.astype(np.float16).astype(np.float32)
        # W pooling: s1[j] = blur[2j] + blur[2j+1]; s2[j] = blur[2j-1] + blur[2j+2]
        bz = np.zeros((s.M, W + 2), np.float32)
        bz[:, 1:1 + W] = blur
        s1 = bz[:, 1:1 + W:2] + bz[:, 2:2 + W:2]
        s2 = bz[:, 0:2 * WO:2] + bz[:, 3::2]
        wp4 = 3.0 * s1 + s2
        if fp16:
            wp4 = wp4.astype(np.float16).astype(np.float32)
        php = s.Pmat.T @ wp4
        out[s.o0:s.o0 + s.P] = php
    # host edge renormalization
    out[0, :] *= 8.0 / 7.0
    out[-1, :] *= 8.0 / 7.0
    out[:, 0] *= 8.0 / 7.0
    out[:, -1] *= 8.0 / 7.0
    return out


# ---------------------------------------------------------------- bass build
def build_nc(n_img, H, W, repeat=1):
    import concourse.bacc as bacc
    import concourse.tile as tile
    import concourse.mybir as mybir
    from contextlib import ExitStack

    f32 = mybir.dt.float32
    f16 = mybir.dt.float16
    AF = mybir.ActivationFunctionType
    OP = mybir.AluOpType

    HP, WP = H + 4, W + 4
    HO, WO = H // 2, W // 2
    slabs = slab_plan(H)
    wts_np = build_weight_arrays(slabs)
    NHALF = (W + 511) // 512

    nc = bacc.Bacc("TRN2", target_bir_lowering=False, debug=False)
    xp = nc.dram_tensor("xp", [n_img, HP, WP], f16, kind="ExternalInput").ap()
    mp = nc.dram_tensor("mp", [n_img, HP, WP], f16, kind="ExternalInput").ap()
    wt_dram = {}
    for i, (key, (warr, parr)) in enumerate(wts_np.items()):
        wt_dram[key] = (
            nc.dram_tensor(f"wt{i}", list(warr.shape), f16,
                           kind="ExternalInput").ap(),
            nc.dram_tensor(f"pm{i}", list(parr.shape), f16,
                           kind="ExternalInput").ap(),
        )
    out = nc.dram_tensor("out", [n_img, HO, WO], f16, kind="ExternalOutput").ap()

    with ExitStack() as ctx:
        tc = ctx.enter_context(tile.TileContext(nc))
        wpool = ctx.enter_context(tc.tile_pool(name="wts", bufs=1))
        wt_sb = {}
        for key, (warr, parr) in wts_np.items():
            i = len(wt_sb)
            wtile = wpool.tile(list(warr.shape), f16, tag=f"w{i}")
            nc.sync.dma_start(wtile[:], wt_dram[key][0])
            ptile = wpool.tile(list(parr.shape), f16, tag=f"p{i}")
            nc.sync.dma_start(ptile[:], wt_dram[key][1])
            wt_sb[key] = (wtile, ptile)

        in_pool = ctx.enter_context(tc.tile_pool(name="inp", bufs=3))
        t_pool = ctx.enter_context(tc.tile_pool(name="tmul", bufs=2))
        ps_pool = ctx.enter_context(tc.tile_pool(name="ps", bufs=3, space="PSUM"))
        pp_pool = ctx.enter_context(tc.tile_pool(name="pp", bufs=2, space="PSUM"))
        rd_pool = ctx.enter_context(tc.tile_pool(name="rd", bufs=2))
        blur_pool = ctx.enter_context(tc.tile_pool(name="blur", bufs=2))
        s_pool = ctx.enter_context(tc.tile_pool(name="spool", bufs=2))
        o_pool = ctx.enter_context(tc.tile_pool(name="opool", bufs=3))

        for img_rep in range(n_img * repeat):
            img = img_rep % n_img
            for s in slabs:
                K, M, P = s.K, s.M, s.P
                wt, pm = wt_sb[s.wkey]
                x_sb = in_pool.tile([K, WP], f16, tag="x")
                sx = xp[img, s.r0:s.r0 + K, :].rearrange("(h e) w -> e h w", e=2)
                nc.sync.dma_start(x_sb[0:K // 2, :], sx[0])
                nc.sync.dma_start(x_sb[K // 2:K, :], sx[1])
                m_sb = in_pool.tile([K, WP], f16, tag="m")
                sm = mp[img, s.r0:s.r0 + K, :].rearrange("(h e) w -> e h w", e=2)
                nc.sync.dma_start(m_sb[0:K // 2, :], sm[0])
                nc.sync.dma_start(m_sb[K // 2:K, :], sm[1])
                t_sb = t_pool.tile([K, WP], f16, tag="t")
                nc.vector.tensor_mul(t_sb[:], x_sb[:], m_sb[:])

                blur = blur_pool.tile([M, W + 2], f16, tag="blur")
                for hf in range(NHALF):
                    w0 = 512 * hf
                    n = min(512, W - w0)
                    pden = ps_pool.tile([M, 512], f32, tag="pd")
                    pnum = ps_pool.tile([M, 512], f32, tag="pn")
                    for tap in range(5):
                        nc.tensor.matmul(
                            pden[:, 0:n], wt[0:K, (5 + tap) * M:(6 + tap) * M],
                            m_sb[0:K, w0 + tap:w0 + tap + n],
                            start=(tap == 0), stop=(tap == 4),
                        )
                    for tap in range(5):
                        nc.tensor.matmul(
                            pnum[:, 0:n], wt[0:K, tap * M:(tap + 1) * M],
                            t_sb[0:K, w0 + tap:w0 + tap + n],
                            start=(tap == 0), stop=(tap == 4),
                        )
                    rden = rd_pool.tile([M, 512], f32, tag="rden")
                    nc.vector.reciprocal_approx_fast(rden[:, 0:n], pden[:, 0:n])
                    nc.vector.tensor_mul(
                        blur[:, 1 + w0:1 + w0 + n], pnum[:, 0:n], rden[:, 0:n]
                    )
                nc.gpsimd.memset(blur[:, 0:1], 0.0)
                nc.gpsimd.memset(blur[:, 1 + W:2 + W], 0.0)

                # W pooling. full[:, a, 0] = blur col 2a-1, full[:, a, 1] = 2a
                # (blur image col c lives at tile col c+1).
                full = blur[:, 0:W + 2].rearrange("p (a t) -> p a t", t=2)
                s1 = s_pool.tile([M, WO], f16, tag="s1")
                nc.gpsimd.tensor_add(s1[:], full[:, 0:WO, 1], full[:, 1:WO + 1, 0])
                s2 = s_pool.tile([M, WO], f16, tag="s2")
                nc.gpsimd.tensor_add(s2[:], full[:, 0:WO, 0], full[:, 1:WO + 1, 1])
                wp4 = s_pool.tile([M, WO], f16, tag="wp4")
                nc.vector.scalar_tensor_tensor(
                    wp4[:], s1[:], 3.0, s2[:], OP.mult, OP.add
                )
                # H pooling via PE
                php = pp_pool.tile([P, WO], f32, tag="php")
                nc.tensor.matmul(php[:], pm[:], wp4[:], start=True, stop=True)
                osb = o_pool.tile([P, WO], f16, tag="osb")
                nc.scalar.activation(osb[:], php[:], AF.Copy)
                nc.sync.dma_start(out[img, s.o0:s.o0 + P, :], osb[:])

    nc.compile()
    return nc, wts_np


# ---------------------------------------------------------------- entry point
_CACHE = {}


def _get_nc(n_img, H, W):
    key = (n_img, H, W)
    if key not in _CACHE:
        _CACHE[key] = build_nc(n_img, H, W)
    return _CACHE[key]


def host_edge_fixup(out):
    """Renormalize image-border pooled rows/cols (3-tap /7 instead of /8)."""
    out[..., 0, :] *= np.float32(8.0 / 7.0)
    out[..., -1, :] *= np.float32(8.0 / 7.0)
    out[..., :, 0] *= np.float32(8.0 / 7.0)
    out[..., :, -1] *= np.float32(8.0 / 7.0)
    return out


def run_on_hw(x_imgs, m_imgs, n_cores=None, trace=False):
    """x_imgs, m_imgs: [n_total_img, H, W] fp32. Shards image dim across cores.
    Returns ([n_total_img, H/2, W/2] fp32, BassKernelResults)."""
    from concourse.bass_utils import run_bass_kernel_spmd

    n_total, h, w = x_imgs.shape
    if n_cores is None:
        n_cores = N_CORES
    assert n_total % n_cores == 0
    n_img = n_total // n_cores
    nc, wts_np = _get_nc(n_img, h, w)

    xp_all = np.pad(x_imgs, ((0, 0), (2, 2), (2, 2)), mode="reflect").astype(np.float16)
    mp_all = np.pad(m_imgs, ((0, 0), (2, 2), (2, 2)), mode="reflect").astype(np.float16)
    in_maps = []
    for c in range(n_cores):
        im = {
            "xp": np.ascontiguousarray(xp_all[c * n_img:(c + 1) * n_img]),
            "mp": np.ascontiguousarray(mp_all[c * n_img:(c + 1) * n_img]),
        }
        for i, (warr, parr) in enumerate(wts_np.values()):
            im[f"wt{i}"] = warr
            im[f"pm{i}"] = parr
        in_maps.append(im)

    res = run_bass_kernel_spmd(nc, in_maps, list(range(n_cores)), trace=trace)
    outs = [r["out"].astype(np.float32) for r in res.results]
    full = np.concatenate(outs, axis=0)
    return host_edge_fixup(full), res


def kernel(input, mask):
    """Full-problem entry point: input/mask [16,3,1024,1024] fp32 ->
    [16,3,512,512] fp32."""
    x = np.asarray(input, np.float32).reshape(B * C, H, W)
    m = np.asarray(mask, np.float32).reshape(B * C, H, W)
    out, _ = run_on_hw(x, m)
    return out.reshape(B, C, H // 2, W // 2)


# revision 24
# speedup vs baseline: 3.6336x; 3.6336x over previous
"""Trainium2 Bass kernel for nn_PyrDown (masked 5x5 Gaussian blur + antialiased
2x bilinear downsample), data-parallel over batch across 8 NeuronCores.

Math per image (reflect-pad 2):
  num = conv2d_5x5(x*m), den = conv2d_5x5(m)  (unnormalized integer binomial
  weights; den weights additionally scaled by 64 to fold the pooling /8/8)
  blur = num * (1/den64)
  out  = antialiased 2x downsample (jax.image.resize antialias=True):
         separable [1,3,3,1]/8 stride 2 with edge renormalization (the /8 is
         folded into den64, edge /7-renorm applied on host as x8/7)

Device mapping per H-slab (M blur rows = P pooled rows * 2 + halo, K = M+4
input rows, even blur rows in low partitions / odd in high -- the DMA loads
rows even/odd-interleaved and the banded weight matrices absorb the
permutation):
  - 4 fp16 DMA loads (x evens/odds, m evens/odds)
  - DVE: t = x*m (fp16)
  - PE: direct 2D conv: per 512-col half, 5 accumulating fp16 matmuls each for
    num and den (H-taps in the banded stationary weights, W-taps via shifted
    rhs slices)
  - DVE: rden = reciprocal_approx_fast(den) (fp32), blur = num*rden (fp16)
  - GPS: W-pool pair sums s1, s2 (stride-2 fp16); DVE: wp4 = 3*s1 + s2
  - PE: H-pool matmul php = Pmat^T @ wp4 ([1,3,3,1] banded, stride-2)
  - ACT: fp16 evac of php; DMA out
"""
import sys
import numpy as np

_TRN_REPO = "/opt/trn_rl_repo"
if _TRN_REPO not in sys.path:
    sys.path.insert(0, _TRN_REPO)

# Problem shape (hardcoded per spec)
B, C, H, W = 16, 3, 1024, 1024
N_CORES = 8

KH = np.array([1.0, 4.0, 6.0, 4.0, 1.0], np.float32)


# ---------------------------------------------------------------- slab plan
class Slab:
    pass


def slab_plan(H):
    """Split the image into H-slabs. Each slab computes P pooled rows, which
    need M blur rows (M even, includes +-1 halo), which need K = M+4 padded
    rows. Partition layout of blur rows: even rows first, then odd rows."""
    HO = H // 2
    slabs = []
    o0 = 0
    while o0 < HO:
        P = min(61, HO - o0)
        b_lo = max(0, 2 * o0 - 1)
        b_hi = min(H - 1, 2 * (o0 + P - 1) + 2)
        if (b_hi - b_lo + 1) % 2:
            if b_lo > 0:
                b_lo -= 1
            else:
                b_hi += 1
                assert b_hi <= H - 1
        M = b_hi - b_lo + 1
        K = M + 4
        assert K <= 128 and b_lo + K <= H + 4
        evens = [b for b in range(b_lo, b_lo + M) if b % 2 == 0]
        odds = [b for b in range(b_lo, b_lo + M) if b % 2 == 1]
        out_rows = evens + odds  # partition order of blur rows
        pos = {r: i for i, r in enumerate(out_rows)}

        s = Slab()
        s.o0, s.P, s.b_lo, s.M, s.K = o0, P, b_lo, M, K
        s.r0 = b_lo  # first padded row to load
        s.out_rows = out_rows
        # input padded rows, partition order (even/odd offset interleave)
        s.in_rows = [s.r0 + 2 * p for p in range(K // 2)] + [
            s.r0 + 2 * p + 1 for p in range(K // 2)
        ]
        # H-pool matrix [M, P]: pooled row o0+q <- blur rows 2o-1,2o,2o+1,2o+2
        # with weights 1,3,3,1; out-of-range taps dropped (host renormalizes
        # the image's first/last pooled row by 8/7).
        Pm = np.zeros((M, P), np.float32)
        for q in range(P):
            o = o0 + q
            for (r, w) in [(2 * o - 1, 1.0), (2 * o, 3.0),
                           (2 * o + 1, 3.0), (2 * o + 2, 1.0)]:
                if 0 <= r <= H - 1:
                    Pm[pos[r], q] = w
        s.Pmat = Pm
        slabs.append(s)
        o0 += P
    return slabs


def build_weight_arrays(slabs):
    """Per distinct slab geometry: conv weights [K, 10*M] fp16 (5 W-taps for
    num, then 5 scaled by 64 for den) and the H-pool matrix [M, P] fp16."""
    arrays = {}
    for s in slabs:
        key = (s.K, s.M, s.o0 == 0, tuple(r - s.r0 for r in s.out_rows))
        s.wkey = key
        if key in arrays:
            continue
        Wt = np.zeros((s.K, 10 * s.M), np.float32)
        for p in range(s.K):
            ir = s.in_rows[p]
            for m in range(s.M):
                d = ir - s.out_rows[m]
                if 0 <= d <= 4:
                    for tap in range(5):
                        Wt[p, tap * s.M + m] = KH[tap] * KH[d]
                        Wt[p, (5 + tap) * s.M + m] = 64.0 * KH[tap] * KH[d]
        arrays[key] = (Wt.astype(np.float16), s.Pmat.astype(np.float16))
    return arrays


# ---------------------------------------------------------------- numpy model
def numpy_model(x, m, H, W, fp16=True):
    """Model of exactly what the device computes (optionally with fp16
    rounding of the conv inputs), plus the host edge fixup."""
    slabs = slab_plan(H)
    WO = W // 2
    xp = np.pad(x, 2, mode="reflect")
    mp = np.pad(m, 2, mode="reflect")
    if fp16:
        xp = xp.astype(np.float16).astype(np.float32)
        mp = mp.astype(np.float16).astype(np.float32)
    tp = (xp * mp)
    if fp16:
        tp = tp.astype(np.float16).astype(np.float32)
    out = np.zeros((H // 2, WO), np.float32)
    for s in slabs:
        t_sb = tp[s.in_rows, :]
        m_sb = mp[s.in_rows, :]
        Wt = np.zeros((s.K, 10 * s.M), np.float32)
        for p in range(s.K):
            ir = s.in_rows[p]
            for mm in range(s.M):
                d = ir - s.out_rows[mm]
                if 0 <= d <= 4:
                    for tap in range(5):
                        Wt[p, tap * s.M + mm] = KH[tap] * KH[d]
                        Wt[p, (5 + tap) * s.M + mm] = 64.0 * KH[tap] * KH[d]
        pnum = np.zeros((s.M, W), np.float32)
        pden = np.zeros((s.M, W), np.float32)
        for tap in range(5):
            pnum += Wt[:, tap * s.M:(tap + 1) * s.M].T @ t_sb[:, tap:tap + W]
            pden += Wt[:, (5 + tap) * s.M:(6 + tap) * s.M].T @ m_sb[:, tap:tap + W]
        rden = 1.0 / pden
        blur = pnum * rden
        if fp16:
            blur = blur.astype(np.float16).astype(np.float32)
        # W pooling: s1[j] = blur[2j] + blur[2j+1]; s2[j] = blur[2j-1] + blur[2j+2]
        bz = np.zeros((s.M, W + 2), np.float32)
        bz[:, 1:1 + W] = blur
        s1 = bz[:, 1:1 + W:2] + bz[:, 2:2 + W:2]
        s2 = bz[:, 0:2 * WO:2] + bz[:, 3::2]
        wp4 = 3.0 * s1 + s2
        if fp16:
            wp4 = wp4.astype(np.float16).astype(np.float32)
        php = s.Pmat.T @ wp4
        out[s.o0:s.o0 + s.P] = php
    # host edge renormalization
    out[0, :] *= 8.0 / 7.0
    out[-1, :] *= 8.0 / 7.0
    out[:, 0] *= 8.0 / 7.0
    out[:, -1] *= 8.0 / 7.0
    return out


# ------------------------------------------------------- v6: separable conv
# Pass 1 (per h-slab x w-chunk): data-as-stationary-weights matmul embeds the
# transpose: out1[w, h'] = sum_h t[h, w] * A[h, h'] -- the H-direction conv.
# Pass 2 (per w-chunk): banded B as stationary weights contracts w:
# out2[w', h'] = sum_w out1[w, h'] * B[w, w'] -- the W-direction conv, result
# stays transposed. Division, pooling all run transposed; host transposes the
# final [WO, HO] image back.
def v6_wchunk_plan(W):
    """W-chunks follow the same pooled-halo geometry as slab_plan: each chunk
    owns P pooled w'-rows, needs M blur w-cols (+-1 halo), K = M+4 input
    cols."""
    return slab_plan(W)


def v6_hslab_plan(H):
    """Plain conv tiling of the h' (free) dimension: 124 outputs per 128-row
    input slab."""
    slabs = []
    h0 = 0
    while h0 < H:
        M = min(124, H - h0)
        slabs.append((h0, M + 4, M))  # (first h', K input rows, M outputs)
        h0 += M
    return slabs


def build_weight_arrays_v6(H, W):
    """A: pass-1 band [128, 124] kh[p-m]; per w-chunk geometry: B band
    [K, M] kw[p-m] for num, B64 for den, and the w'-pool matrix [M, P]."""
    A = np.zeros((128, 124), np.float32)
    for p in range(128):
        for m in range(124):
            if 0 <= p - m <= 4:
                A[p, m] = KH[p - m]
    wchunks = v6_wchunk_plan(W)
    arrays = {"A": A.astype(np.float16)}
    for s in wchunks:
        key = (s.K, s.M, s.o0 == 0)
        s.wkey = key
        if f"B{key}" in arrays:
            continue
        Bm = np.zeros((s.K, 2 * s.M), np.float32)
        for p in range(s.K):
            for m in range(s.M):
                if 0 <= p - m <= 4:
                    Bm[p, m] = KH[p - m]
                    Bm[p, s.M + m] = 64.0 * KH[p - m]
        Pm = np.zeros((s.M, s.P), np.float32)
        for q in range(s.P):
            o = s.o0 + q
            for (r, w) in [(2 * o - 1, 1.0), (2 * o, 3.0),
                           (2 * o + 1, 3.0), (2 * o + 2, 1.0)]:
                if 0 <= r <= W - 1 and 0 <= r - s.b_lo < s.M:
                    Pm[r - s.b_lo, q] = w
        arrays[f"B{key}"] = (Bm.astype(np.float16), Pm.astype(np.float16))
    return arrays, wchunks


def numpy_model_v6(x, m, H, W, fp16=True):
    """Mirror of the v6 device computation + host transpose/edge fixup."""
    arrays, wchunks = build_weight_arrays_v6(H, W)
    A = arrays["A"].astype(np.float32)
    hslabs = v6_hslab_plan(H)
    xp = np.pad(x, 2, mode="reflect")
    mp = np.pad(m, 2, mode="reflect")
    if fp16:
        xp = xp.astype(np.float16).astype(np.float32)
        mp = mp.astype(np.float16).astype(np.float32)
    tp = xp * mp
    if fp16:
        tp = tp.astype(np.float16).astype(np.float32)
    HO, WO = H // 2, W // 2
    outT = np.zeros((WO, HO), np.float32)
    for s in wchunks:
        Bm, Pm = arrays[f"B{s.wkey}"]
        Bm = Bm.astype(np.float32)
        Pm = Pm.astype(np.float32)
        out1 = {}
        for fld, src in (("t", tp), ("m", mp)):
            o1 = np.zeros((s.K, H), np.float32)  # [w, h']
            for (h0, Kh, Mh) in hslabs:
                chunk = src[h0:h0 + Kh, s.r0:s.r0 + s.K]  # [h, w]
                o1[:, h0:h0 + Mh] = (chunk.T @ A[:Kh, :Mh]).astype(
                    np.float16).astype(np.float32) if fp16 else chunk.T @ A[:Kh, :Mh]
            out1[fld] = o1
        numT = Bm[:, :s.M].T @ out1["t"]          # [w', h']
        denT = Bm[:, s.M:].T @ out1["m"]
        blurT = numT * (1.0 / denT)
        if fp16:
            blurT = blurT.astype(np.float16).astype(np.float32)
        # pool along h' (free dim): s1/s2 with zero edges
        bz = np.zeros((s.M, H + 2), np.float32)
        bz[:, 1:1 + H] = blurT
        s1 = bz[:, 1:1 + H:2] + bz[:, 2:2 + H:2]
        s2 = bz[:, 0:2 * HO:2] + bz[:, 3::2]
        wp4 = 3.0 * s1 + s2
        if fp16:
            wp4 = wp4.astype(np.float16).astype(np.float32)
        outT[s.o0:s.o0 + s.P] = Pm.T @ wp4
    out = outT.T.copy()
    out[0, :] *= 8.0 / 7.0
    out[-1, :] *= 8.0 / 7.0
    out[:, 0] *= 8.0 / 7.0
    out[:, -1] *= 8.0 / 7.0
    return out


def build_nc_v6(n_img, H, W, repeat=1):
    import concourse.bacc as bacc
    import concourse.tile as tile
    import concourse.mybir as mybir
    from contextlib import ExitStack

    f32 = mybir.dt.float32
    f16 = mybir.dt.float16
    AF = mybir.ActivationFunctionType
    OP = mybir.AluOpType

    HP, WP = H + 4, W + 4
    HO, WO = H // 2, W // 2
    arrays, wchunks = build_weight_arrays_v6(H, W)
    hslabs = v6_hslab_plan(H)
    n_hs = len(hslabs)
    # pass-1 psum grouping: batches of h-slabs, <=512 output cols per bank
    groups = []
    g, w = [], 0
    for i in range(n_hs):
        Mh = hslabs[i][2]
        if w + Mh > 512:
            groups.append(g)
            g, w = [], 0
        g.append(i)
        w += Mh
    if g:
        groups.append(g)

    nc = bacc.Bacc("TRN2", target_bir_lowering=False, debug=False)
    xp = nc.dram_tensor("xp", [n_img, HP, WP], f16, kind="ExternalInput").ap()
    mp = nc.dram_tensor("mp", [n_img, HP, WP], f16, kind="ExternalInput").ap()
    wa_dram = nc.dram_tensor("wA", list(arrays["A"].shape), f16,
                             kind="ExternalInput").ap()
    wt_dram = {}
    wkeys = [k for k in arrays if k != "A"]
    for i, k in enumerate(wkeys):
        Bm, Pm = arrays[k]
        wt_dram[k] = (
            nc.dram_tensor(f"wB{i}", list(Bm.shape), f16, kind="ExternalInput").ap(),
            nc.dram_tensor(f"wP{i}", list(Pm.shape), f16, kind="ExternalInput").ap(),
        )
    # output is transposed: [WO, HO] per image
    out = nc.dram_tensor("out", [n_img, WO, HO], f16, kind="ExternalOutput").ap()

    with ExitStack() as ctx:
        tc = ctx.enter_context(tile.TileContext(nc))
        wpool = ctx.enter_context(tc.tile_pool(name="wts", bufs=1))
        A_sb = wpool.tile(list(arrays["A"].shape), f16, tag="A")
        nc.sync.dma_start(A_sb[:], wa_dram)
        wt_sb = {}
        for i, k in enumerate(wkeys):
            Bm, Pm = arrays[k]
            bt = wpool.tile(list(Bm.shape), f16, tag=f"B{i}")
            nc.sync.dma_start(bt[:], wt_dram[k][0])
            pt = wpool.tile(list(Pm.shape), f16, tag=f"P{i}")
            nc.sync.dma_start(pt[:], wt_dram[k][1])
            wt_sb[k] = (bt, pt)

        x_pool = ctx.enter_context(tc.tile_pool(name="xp", bufs=2))
        # m and t slab sets persist per image; 2 image-sets for overlap
        mt_pool = ctx.enter_context(tc.tile_pool(name="mt", bufs=2))
        ps1_pool = ctx.enter_context(tc.tile_pool(name="ps1", bufs=2, space="PSUM"))
        o1_pool = ctx.enter_context(tc.tile_pool(name="o1", bufs=2))
        ps2_pool = ctx.enter_context(tc.tile_pool(name="ps2", bufs=1, space="PSUM"))
        rd_pool = ctx.enter_context(tc.tile_pool(name="rd", bufs=2))
        blur_pool = ctx.enter_context(tc.tile_pool(name="blur", bufs=2))
        s_pool = ctx.enter_context(tc.tile_pool(name="spool", bufs=2))
        pp_pool = ctx.enter_context(tc.tile_pool(name="pp", bufs=2, space="PSUM"))
        o_pool = ctx.enter_context(tc.tile_pool(name="opool", bufs=3))

        for img_rep in range(n_img * repeat):
            img = img_rep % n_img
            # load m slabs, compute t slabs (full width, fp16)
            m_tiles, t_tiles = [], []
            for hi, (h0, Kh, Mh) in enumerate(hslabs):
                x_sb = x_pool.tile([Kh, WP], f16, tag="x")
                nc.sync.dma_start(x_sb[:], xp[img, h0:h0 + Kh, :])
                m_sb = mt_pool.tile([Kh, WP], f16, tag=f"m{hi}")
                nc.sync.dma_start(m_sb[:], mp[img, h0:h0 + Kh, :])
                t_sb = mt_pool.tile([Kh, WP], f16, tag=f"t{hi}")
                nc.vector.tensor_mul(t_sb[:], x_sb[:], m_sb[:])
                m_tiles.append(m_sb)
                t_tiles.append(t_sb)

            for s in wchunks:
                Kw, Mw, P = s.K, s.M, s.P
                bt, pt = wt_sb[f"B{s.wkey}"]
                out1 = {}
                for fld, tiles in (("t", t_tiles), ("m", m_tiles)):
                    o1 = o1_pool.tile([Kw, H], f16, tag=f"o1{fld}")
                    for grp in groups:
                        g_h0 = hslabs[grp[0]][0]
                        g_w = sum(hslabs[i][2] for i in grp)
                        ps = ps1_pool.tile([Kw, 512], f32, tag="ps1")
                        for gi in grp:
                            h0, Kh, Mh = hslabs[gi]
                            c0 = h0 - g_h0
                            nc.tensor.matmul(
                                ps[:, c0:c0 + Mh],
                                tiles[gi][0:Kh, s.r0:s.r0 + Kw],
                                A_sb[0:Kh, 0:Mh],
                                start=True, stop=True,
                            )
                        # evac group -> fp16 (ACT is the cheapest PSUM reader)
                        nc.scalar.activation(
                            o1[:, g_h0:g_h0 + g_w], ps[:, 0:g_w], AF.Copy
                        )
                    out1[fld] = o1
                # pass 2: both fields, both halves, into 2-bank psum tiles
                pnum = ps2_pool.tile([Mw, H], f32, tag="pn")
                pden = ps2_pool.tile([Mw, H], f32, tag="pd")
                for h0 in range(0, H, 512):
                    n = min(512, H - h0)
                    nc.tensor.matmul(
                        pden[:, h0:h0 + n], bt[0:Kw, Mw:2 * Mw],
                        out1["m"][0:Kw, h0:h0 + n], start=True, stop=True,
                    )
                    nc.tensor.matmul(
                        pnum[:, h0:h0 + n], bt[0:Kw, 0:Mw],
                        out1["t"][0:Kw, h0:h0 + n], start=True, stop=True,
                    )
                rden = rd_pool.tile([Mw, H], f32, tag="rden")
                nc.vector.reciprocal_approx_fast(rden[:], pden[:])
                blur = blur_pool.tile([Mw, H + 2], f16, tag="blur")
                nc.vector.tensor_mul(blur[:, 1:1 + H], pnum[:], rden[:])
                nc.gpsimd.memset(blur[:, 0:1], 0.0)
                nc.gpsimd.memset(blur[:, 1 + H:2 + H], 0.0)
                # pool along h' (free dim)
                full = blur[:, 0:H + 2].rearrange("p (a t) -> p a t", t=2)
                s1 = s_pool.tile([Mw, HO], f16, tag="s1")
                nc.gpsimd.tensor_add(s1[:], full[:, 0:HO, 1], full[:, 1:HO + 1, 0])
                s2 = s_pool.tile([Mw, HO], f16, tag="s2")
                nc.gpsimd.tensor_add(s2[:], full[:, 0:HO, 0], full[:, 1:HO + 1, 1])
                wp4 = s_pool.tile([Mw, HO], f16, tag="wp4")
                nc.vector.scalar_tensor_tensor(
                    wp4[:], s1[:], 3.0, s2[:], OP.mult, OP.add
                )
                # pool along w' (partition dim) via PE
                php = pp_pool.tile([P, HO], f32, tag="php")
                nc.tensor.matmul(php[:], pt[:], wp4[:], start=True, stop=True)
                osb = o_pool.tile([P, HO], f16, tag="osb")
                nc.scalar.activation(osb[:], php[:], AF.Copy)
                nc.sync.dma_start(out[img, s.o0:s.o0 + P, :], osb[:])

    nc.compile()
    return nc, arrays, wkeys


# ---------------------------------------------------------------- bass build
def build_nc(n_img, H, W, repeat=1):
    import concourse.bacc as bacc
    import concourse.tile as tile
    import concourse.mybir as mybir
    from contextlib import ExitStack

    f32 = mybir.dt.float32
    f16 = mybir.dt.float16
    AF = mybir.ActivationFunctionType
    OP = mybir.AluOpType

    HP, WP = H + 4, W + 4
    HO, WO = H // 2, W // 2
    slabs = slab_plan(H)
    wts_np = build_weight_arrays(slabs)
    NHALF = (W + 511) // 512

    nc = bacc.Bacc("TRN2", target_bir_lowering=False, debug=False)
    xp = nc.dram_tensor("xp", [n_img, HP, WP], f16, kind="ExternalInput").ap()
    mp = nc.dram_tensor("mp", [n_img, HP, WP], f16, kind="ExternalInput").ap()
    wt_dram = {}
    for i, (key, (warr, parr)) in enumerate(wts_np.items()):
        wt_dram[key] = (
            nc.dram_tensor(f"wt{i}", list(warr.shape), f16,
                           kind="ExternalInput").ap(),
            nc.dram_tensor(f"pm{i}", list(parr.shape), f16,
                           kind="ExternalInput").ap(),
        )
    out = nc.dram_tensor("out", [n_img, HO, WO], f16, kind="ExternalOutput").ap()

    with ExitStack() as ctx:
        tc = ctx.enter_context(tile.TileContext(nc))
        wpool = ctx.enter_context(tc.tile_pool(name="wts", bufs=1))
        wt_sb = {}
        for key, (warr, parr) in wts_np.items():
            i = len(wt_sb)
            wtile = wpool.tile(list(warr.shape), f16, tag=f"w{i}")
            nc.sync.dma_start(wtile[:], wt_dram[key][0])
            ptile = wpool.tile(list(parr.shape), f16, tag=f"p{i}")
            nc.sync.dma_start(ptile[:], wt_dram[key][1])
            wt_sb[key] = (wtile, ptile)

        in_pool = ctx.enter_context(tc.tile_pool(name="inp", bufs=3))
        t_pool = ctx.enter_context(tc.tile_pool(name="tmul", bufs=2))
        ps_pool = ctx.enter_context(tc.tile_pool(name="ps", bufs=3, space="PSUM"))
        pp_pool = ctx.enter_context(tc.tile_pool(name="pp", bufs=2, space="PSUM"))
        rd_pool = ctx.enter_context(tc.tile_pool(name="rd", bufs=2))
        blur_pool = ctx.enter_context(tc.tile_pool(name="blur", bufs=2))
        s_pool = ctx.enter_context(tc.tile_pool(name="spool", bufs=2))
        o_pool = ctx.enter_context(tc.tile_pool(name="opool", bufs=3))

        for img_rep in range(n_img * repeat):
            img = img_rep % n_img
            for s in slabs:
                K, M, P = s.K, s.M, s.P
                wt, pm = wt_sb[s.wkey]
                x_sb = in_pool.tile([K, WP], f16, tag="x")
                sx = xp[img, s.r0:s.r0 + K, :].rearrange("(h e) w -> e h w", e=2)
                nc.sync.dma_start(x_sb[0:K // 2, :], sx[0])
                nc.sync.dma_start(x_sb[K // 2:K, :], sx[1])
                m_sb = in_pool.tile([K, WP], f16, tag="m")
                sm = mp[img, s.r0:s.r0 + K, :].rearrange("(h e) w -> e h w", e=2)
                nc.sync.dma_start(m_sb[0:K // 2, :], sm[0])
                nc.sync.dma_start(m_sb[K // 2:K, :], sm[1])
                t_sb = t_pool.tile([K, WP], f16, tag="t")
                nc.vector.tensor_mul(t_sb[:], x_sb[:], m_sb[:])

                blur = blur_pool.tile([M, W + 2], f16, tag="blur")
                for hf in range(NHALF):
                    w0 = 512 * hf
                    n = min(512, W - w0)
                    pden = ps_pool.tile([M, 512], f32, tag="pd")
                    pnum = ps_pool.tile([M, 512], f32, tag="pn")
                    for tap in range(5):
                        nc.tensor.matmul(
                            pden[:, 0:n], wt[0:K, (5 + tap) * M:(6 + tap) * M],
                            m_sb[0:K, w0 + tap:w0 + tap + n],
                            start=(tap == 0), stop=(tap == 4),
                        )
                    for tap in range(5):
                        nc.tensor.matmul(
                            pnum[:, 0:n], wt[0:K, tap * M:(tap + 1) * M],
                            t_sb[0:K, w0 + tap:w0 + tap + n],
                            start=(tap == 0), stop=(tap == 4),
                        )
                    rden = rd_pool.tile([M, 512], f32, tag="rden")
                    nc.vector.reciprocal_approx_fast(rden[:, 0:n], pden[:, 0:n])
                    nc.vector.tensor_mul(
                        blur[:, 1 + w0:1 + w0 + n], pnum[:, 0:n], rden[:, 0:n]
                    )
                nc.gpsimd.memset(blur[:, 0:1], 0.0)
                nc.gpsimd.memset(blur[:, 1 + W:2 + W], 0.0)

                # W pooling. full[:, a, 0] = blur col 2a-1, full[:, a, 1] = 2a
                # (blur image col c lives at tile col c+1).
                full = blur[:, 0:W + 2].rearrange("p (a t) -> p a t", t=2)
                s1 = s_pool.tile([M, WO], f16, tag="s1")
                nc.gpsimd.tensor_add(s1[:], full[:, 0:WO, 1], full[:, 1:WO + 1, 0])
                s2 = s_pool.tile([M, WO], f16, tag="s2")
                nc.gpsimd.tensor_add(s2[:], full[:, 0:WO, 0], full[:, 1:WO + 1, 1])
                wp4 = s_pool.tile([M, WO], f16, tag="wp4")
                nc.vector.scalar_tensor_tensor(
                    wp4[:], s1[:], 3.0, s2[:], OP.mult, OP.add
                )
                # H pooling via PE
                php = pp_pool.tile([P, WO], f32, tag="php")
                nc.tensor.matmul(php[:], pm[:], wp4[:], start=True, stop=True)
                osb = o_pool.tile([P, WO], f16, tag="osb")
                nc.scalar.activation(osb[:], php[:], AF.Copy)
                nc.sync.dma_start(out[img, s.o0:s.o0 + P, :], osb[:])

    nc.compile()
    return nc, wts_np


# ---------------------------------------------------------------- entry point
_CACHE = {}


def _get_nc(n_img, H, W):
    key = (n_img, H, W)
    if key not in _CACHE:
        _CACHE[key] = build_nc(n_img, H, W)
    return _CACHE[key]


def host_edge_fixup(out):
    """Renormalize image-border pooled rows/cols (3-tap /7 instead of /8)."""
    out[..., 0, :] *= np.float32(8.0 / 7.0)
    out[..., -1, :] *= np.float32(8.0 / 7.0)
    out[..., :, 0] *= np.float32(8.0 / 7.0)
    out[..., :, -1] *= np.float32(8.0 / 7.0)
    return out


def make_in_maps_v6(xp_all, mp_all, arrays, wkeys, n_cores, n_img):
    in_maps = []
    for c in range(n_cores):
        im = {
            "xp": np.ascontiguousarray(xp_all[c * n_img:(c + 1) * n_img]),
            "mp": np.ascontiguousarray(mp_all[c * n_img:(c + 1) * n_img]),
            "wA": arrays["A"],
        }
        for i, k in enumerate(wkeys):
            im[f"wB{i}"] = arrays[k][0]
            im[f"wP{i}"] = arrays[k][1]
        in_maps.append(im)
    return in_maps


def run_on_hw(x_imgs, m_imgs, n_cores=None, trace=False):
    """x_imgs, m_imgs: [n_total_img, H, W] fp32. Shards image dim across cores.
    Returns ([n_total_img, H/2, W/2] fp32, BassKernelResults)."""
    from concourse.bass_utils import run_bass_kernel_spmd

    n_total, h, w = x_imgs.shape
    if n_cores is None:
        n_cores = N_CORES
    assert n_total % n_cores == 0
    n_img = n_total // n_cores
    nc, wts_np = _get_nc(n_img, h, w)

    xp_all = np.pad(x_imgs, ((0, 0), (2, 2), (2, 2)), mode="reflect").astype(np.float16)
    mp_all = np.pad(m_imgs, ((0, 0), (2, 2), (2, 2)), mode="reflect").astype(np.float16)
    in_maps = []
    for c in range(n_cores):
        im = {
            "xp": np.ascontiguousarray(xp_all[c * n_img:(c + 1) * n_img]),
            "mp": np.ascontiguousarray(mp_all[c * n_img:(c + 1) * n_img]),
        }
        for i, (warr, parr) in enumerate(wts_np.values()):
            im[f"wt{i}"] = warr
            im[f"pm{i}"] = parr
        in_maps.append(im)

    res = run_bass_kernel_spmd(nc, in_maps, list(range(n_cores)), trace=trace)
    outs = [r["out"].astype(np.float32) for r in res.results]
    full = np.concatenate(outs, axis=0)
    return host_edge_fixup(full), res


def kernel(input, mask):
    """Full-problem entry point: input/mask [16,3,1024,1024] fp32 ->
    [16,3,512,512] fp32."""
    x = np.asarray(input, np.float32).reshape(B * C, H, W)
    m = np.asarray(mask, np.float32).reshape(B * C, H, W)
    out, _ = run_on_hw(x, m)
    return out.reshape(B, C, H // 2, W // 2)
